# revision 1
# baseline (speedup 1.0000x reference)
"""Trainium2 Bass kernel for nn_DeferredRender (4-level bilinear grid_sample sum).

"Mega-entry" single-gather design
---------------------------------
For pixel (u, v), level L uses gx_L = u*W_L - 0.5, x0_L = floor(gx_L) (same
for y). Given the finest-level cell (x0_0, y0_0), each coarser level's x0_L is
confined to {xb_L, xb_L+1} with xb_L = floor((x0_0 - 2^(L-1)) / 2^L) — exact
even at float-rounding edges, because u*1024 = 2^k * (u*W_L) in binary fp. So
a 3x3 super-patch of level L anchored at (yb_L, xb_L) covers every possible
2x2 footprint of the pixel at that level.

The host builds one fp16 table indexed by (r0, k0) = (y0_0+1, x0_0+1):

  entry = [ L0 2x2 patch [dx,dy,c]  :  32 fp16 ]
          [ L1 3x3 patch [dx,dy,c]  :  72 fp16 ]
          [ L2 3x3 patch [dx,dy,c]  :  72 fp16 ]
          [ L3 3x3 patch [dx,dy,c]  :  72 fp16 ]   = 248 fp16 = 496 B

with zeros for out-of-bounds texels, which implements grid_sample's zero
padding for free (no masks or clamps anywhere).

Device kernel (per core, 256 of 2048 rows, H-sharded 8 ways): per [128 x K]
pixel block, compute the L0 cell + per-level fractions on ACT/DVE, fetch one
496B entry per pixel via SWDGE indirect DMA ([128,1] indices per instruction —
the HW-supported form), then weighted-sum: L0 with 4 corner weights, L1-3 with
3-wide zero-stencil weights placed at offset ox_L = x0_L - xb_L in {0,1}.
fp16 MAC, fp32 output, channel-major store.
"""

import numpy as np

C = 8
FULL_H = 2048
FULL_W = 2048
N_CORES = 8
ROWS = FULL_H // N_CORES  # 256
K = 128  # pixels per block column chunk

_CACHED = {}

L0 = 1024
ENT = 248           # fp16 elems per entry
GRID = L0 + 1       # 1025 values of r0/k0


def _build_mega_table(tex0, tex1, tex2, tex3):
    texs = [np.asarray(t, np.float32) for t in (tex0, tex1, tex2, tex3)]
    g = GRID
    x0 = np.arange(-1, L0)  # [-1 .. 1023]
    out = np.zeros((g, g, ENT), np.float16)

    def put(level_tex, base, dst, di, dj):
        H = level_tex.shape[1]
        W = level_tex.shape[2]
        yy = base + di
        xx = base + dj
        yv = (yy >= 0) & (yy < H)
        xv = (xx >= 0) & (xx < W)
        yc = np.clip(yy, 0, H - 1)
        xc = np.clip(xx, 0, W - 1)
        vals = level_tex[:, yc[:, None], xc[None, :]].transpose(1, 2, 0)
        vals = vals * (yv[:, None, None] & xv[None, :, None])
        dst[...] = vals.astype(np.float16)

    v = out[:, :, 0:32].reshape(g, g, 2, 2, C)
    for dx in range(2):
        for dy in range(2):
            put(texs[0], x0, v[:, :, dx, dy, :], dy, dx)
    off = 32
    for li in range(1, 4):
        half = 1 << (li - 1)
        b = (x0 - half) >> li
        v = out[:, :, off:off + 72].reshape(g, g, 3, 3, C)
        for dx in range(3):
            for dy in range(3):
                put(texs[li], b, v[:, :, dx, dy, :], dy, dx)
        off += 72
    return np.ascontiguousarray(out.reshape(g * g, ENT))


def _build_nc(rows, width, kk):
    import concourse.bacc as bacc
    import concourse.bass as bass
    import concourse.mybir as mybir
    import concourse.tile as tile

    f32 = mybir.dt.float32
    f16 = mybir.dt.float16
    i32 = mybir.dt.int32
    Copy = mybir.ActivationFunctionType.Copy
    MUL = mybir.AluOpType.mult
    ADD = mybir.AluOpType.add
    SUB = mybir.AluOpType.subtract

    nc = bacc.Bacc("TRN2", target_bir_lowering=False, debug=False,
                   num_devices=N_CORES)
    u_d = nc.dram_tensor("u", [rows, width], f32, kind="ExternalInput")
    v_d = nc.dram_tensor("v", [rows, width], f32, kind="ExternalInput")
    tbl_d = nc.dram_tensor("tbl", [GRID * GRID, ENT], f16, kind="ExternalInput")
    out_d = nc.dram_tensor("out", [C, rows, width], f32, kind="ExternalOutput")

    with tile.TileContext(nc) as tc:
        with tc.tile_pool(name="main", bufs=2) as pool:
            for r0 in range(0, rows, 128):
                for w0 in range(0, width, kk):
                    u_t = pool.tile([128, kk], f32, tag="u")
                    v_t = pool.tile([128, kk], f32, tag="v")
                    nc.sync.dma_start(u_t[:], u_d.ap()[r0:r0 + 128, w0:w0 + kk])
                    nc.sync.dma_start(v_t[:], v_d.ap()[r0:r0 + 128, w0:w0 + kk])

                    def cell(src, w, tagp):
                        """k = round(u*w) (HW cvt rounds); f = u*w + 0.5 - k."""
                        s = pool.tile([128, kk], f32, tag=f"s{tagp}")
                        nc.scalar.activation(s[:], src[:], Copy,
                                             bias=0.0, scale=float(w))
                        ki = pool.tile([128, kk], i32, tag=f"ki{tagp}")
                        nc.vector.tensor_copy(ki[:], s[:])
                        kf = pool.tile([128, kk], f32, tag=f"kf{tagp}")
                        nc.vector.tensor_copy(kf[:], ki[:])
                        fr = pool.tile([128, kk], f32, tag=f"fr{tagp}")
                        nc.vector.scalar_tensor_tensor(
                            out=fr[:], in0=s[:], scalar=0.5, in1=kf[:],
                            op0=ADD, op1=SUB)
                        return kf, fr

                    kx0, fx0 = cell(u_t, L0, "x0")
                    ky0, fy0 = cell(v_t, L0, "y0")

                    # idx = ky0*GRID + kx0  (kx0/ky0 are already the +1-shifted
                    # grid coords: kx0 = floor(gx)+1)
                    idx = pool.tile([128, kk], i32, tag="idx")
                    nc.vector.scalar_tensor_tensor(
                        out=idx[:], in0=ky0[:], scalar=float(GRID),
                        in1=kx0[:], op0=MUL, op1=ADD)

                    patch = pool.tile([128, kk * ENT], f16, tag="patch")
                    p3 = patch[:].rearrange("p (k e) -> p k e", e=ENT)
                    for k in range(kk):
                        nc.gpsimd.indirect_dma_start(
                            out=p3[:, k, :],
                            out_offset=None,
                            in_=tbl_d.ap(),
                            in_offset=bass.IndirectOffsetOnAxis(
                                ap=idx[:, k:k + 1], axis=0),
                        )

                    acc = pool.tile([128, kk * C], f16, tag="acc")
                    pv = p3

                    # ---- L0: 4-corner MAC ----
                    gx0 = pool.tile([128, kk], f32, tag="gx0")
                    gy0 = pool.tile([128, kk], f32, tag="gy0")
                    nc.scalar.activation(gx0[:], fx0[:], Copy, bias=1.0,
                                         scale=-1.0)
                    nc.scalar.activation(gy0[:], fy0[:], Copy, bias=1.0,
                                         scale=-1.0)
                    w4 = pool.tile([128, 4 * kk], f16, tag="w4")
                    w4v = w4[:].rearrange("p (j k) -> p j k", j=4)
                    nc.vector.tensor_mul(w4v[:, 0, :], gx0[:], gy0[:])
                    nc.vector.tensor_mul(w4v[:, 1, :], gx0[:], fy0[:])
                    nc.vector.tensor_mul(w4v[:, 2, :], fx0[:], gy0[:])
                    nc.vector.tensor_mul(w4v[:, 3, :], fx0[:], fy0[:])
                    w4b = (w4[:].rearrange("p (j k) -> p j k", j=4)
                           .transpose([0, 2, 1]).unsqueeze(3)
                           .broadcast_to([128, kk, 4, C]))
                    l0v = p3[:, :, 0:32].rearrange("p k (j c) -> p k j c", c=C)
                    nc.vector.tensor_mul(l0v, w4b, l0v)
                    nc.vector.tensor_add(pv[:, :, 0:16], pv[:, :, 0:16],
                                         pv[:, :, 16:32])
                    nc.vector.tensor_add(pv[:, :, 0:8], pv[:, :, 0:8],
                                         pv[:, :, 8:16])
                    nc.vector.tensor_copy(acc[:], pv[:, :, 0:8])

                    # ---- L1..L3: 3x3 stencil MAC ----
                    off = 32
                    for li in range(1, 4):
                        half = float(1 << (li - 1))
                        inv = 1.0 / float(1 << li)
                        w3 = {}
                        for coord, src, k0f in (("x", u_t, kx0),
                                                ("y", v_t, ky0)):
                            kLf, frL = cell(src, L0 >> li, coord)
                            # xb = floor((k0 - 1 - half) * inv); bias centers
                            # the dyadic frac grid so round-nearest == floor.
                            bias = -((1.0 + half) * inv) - (0.5 - 0.5 * inv)
                            t = pool.tile([128, kk], f32, tag=f"t{coord}")
                            nc.scalar.activation(t[:], k0f[:], Copy,
                                                 bias=bias, scale=inv)
                            xbi = pool.tile([128, kk], i32, tag=f"xbi{coord}")
                            nc.vector.tensor_copy(xbi[:], t[:])
                            xbf = pool.tile([128, kk], f32, tag=f"xbf{coord}")
                            nc.vector.tensor_copy(xbf[:], xbi[:])
                            # ox = (kL - 1) - xb  in {0, 1}
                            ox = pool.tile([128, kk], f32, tag=f"ox{coord}")
                            nc.vector.scalar_tensor_tensor(
                                out=ox[:], in0=kLf[:], scalar=-1.0,
                                in1=xbf[:], op0=ADD, op1=SUB)
                            # stencil: s0=(1-ox)(1-f), s2=ox*f, s1=1-s0-s2
                            a = pool.tile([128, kk], f32, tag=f"a{coord}")
                            nc.scalar.activation(a[:], frL[:], Copy,
                                                 bias=1.0, scale=-1.0)
                            b = pool.tile([128, kk], f32, tag=f"b{coord}")
                            nc.scalar.activation(b[:], ox[:], Copy,
                                                 bias=1.0, scale=-1.0)
                            s0 = pool.tile([128, kk], f32, tag=f"s0{coord}")
                            nc.vector.tensor_mul(s0[:], b[:], a[:])
                            s2 = pool.tile([128, kk], f32, tag=f"s2{coord}")
                            nc.vector.tensor_mul(s2[:], ox[:], frL[:])
                            sm = pool.tile([128, kk], f32, tag=f"sm{coord}")
                            nc.scalar.activation(sm[:], s0[:], Copy,
                                                 bias=1.0, scale=-1.0)
                            s1 = pool.tile([128, kk], f32, tag=f"s1{coord}")
                            nc.vector.tensor_sub(s1[:], sm[:], s2[:])
                            w3[coord] = (s0, s1, s2)

                        w9 = pool.tile([128, 9 * kk], f16, tag="w9")
                        w9v = w9[:].rearrange("p (j k) -> p j k", j=9)
                        for jx in range(3):
                            for jy in range(3):
                                nc.vector.tensor_mul(
                                    w9v[:, jx * 3 + jy, :],
                                    w3["x"][jx][:], w3["y"][jy][:])
                        w9b = (w9[:].rearrange("p (j k) -> p j k", j=9)
                               .transpose([0, 2, 1]).unsqueeze(3)
                               .broadcast_to([128, kk, 9, C]))
                        lv = p3[:, :, off:off + 72].rearrange(
                            "p k (j c) -> p k j c", c=C)
                        nc.vector.tensor_mul(lv, w9b, lv)
                        o = off
                        nc.vector.tensor_add(pv[:, :, o:o + 24],
                                             pv[:, :, o:o + 24],
                                             pv[:, :, o + 24:o + 48])
                        nc.vector.tensor_add(pv[:, :, o:o + 24],
                                             pv[:, :, o:o + 24],
                                             pv[:, :, o + 48:o + 72])
                        nc.vector.tensor_add(pv[:, :, o:o + 8],
                                             pv[:, :, o:o + 8],
                                             pv[:, :, o + 8:o + 16])
                        nc.vector.tensor_add(pv[:, :, o:o + 8],
                                             pv[:, :, o:o + 8],
                                             pv[:, :, o + 16:o + 24])
                        nc.vector.tensor_add(acc[:], acc[:], pv[:, :, o:o + 8])
                        off += 72

                    accv = acc[:].rearrange("p (k c) -> p k c", c=C)
                    stage = pool.tile([128, kk * C], f32, tag="stage")
                    stv = stage[:].rearrange("p (c k) -> p c k", c=C)
                    for c in range(C):
                        nc.vector.tensor_copy(stv[:, c, :], accv[:, :, c])
                        nc.sync.dma_start(
                            out_d.ap()[c, r0:r0 + 128, w0:w0 + kk],
                            stv[:, c, :])
    nc.compile()
    return nc


def _get_nc(key, *args):
    if key not in _CACHED:
        _CACHED[key] = _build_nc(*args)
    return _CACHED[key]


def kernel(uv_tensor, iter_nr, tex0, tex1, tex2, tex3):
    from concourse import bass_utils

    bass_utils.upload_artifacts = lambda tmpdir: "local://" + tmpdir

    uv = np.asarray(uv_tensor, dtype=np.float32)
    assert uv.shape == (1, 2, FULL_H, FULL_W), uv.shape
    tbl = _build_mega_table(tex0, tex1, tex2, tex3)

    nc = _get_nc("full", ROWS, FULL_W, K)

    in_maps = []
    for i in range(N_CORES):
        r0 = i * ROWS
        in_maps.append({
            "u": np.ascontiguousarray(uv[0, 0, r0:r0 + ROWS, :]),
            "v": np.ascontiguousarray(uv[0, 1, r0:r0 + ROWS, :]),
            "tbl": tbl,
        })

    res = bass_utils.run_bass_kernel_spmd(
        nc, in_maps, core_ids=list(range(N_CORES)))
    globals()["_LAST_RES"] = res
    out = np.concatenate(
        [res.results[i]["out"][None] for i in range(N_CORES)], axis=2)
    return out.astype(np.float32)



# revision 3
# speedup vs baseline: 5.3126x; 5.3126x over previous
"""Trainium2 Bass kernel for nn_DeferredRender (4-level bilinear grid_sample sum).

Collapsed-pyramid single-gather design
--------------------------------------
The reference output f(u,v) = sum of 4 bilinear grid_samples at levels
W_L = 1024 >> L. Each level is piecewise bilinear with breakpoints at
u = (m+0.5)/W_L, and the union of all breakpoints is a subset of the uniform
grid u = j/2048. So f is *exactly* a single bilinear interpolation of the
node grid T[c, jy, jx] = f(jx/2048, jy/2048) sampled at (s,t) = (u,v)*2048.

The host builds T once (separable 1-D upsampling of each level, zero padding
handled there), then packs a fp16 entry table indexed by (y0, x0) = floor(t),
floor(s):

  E[y0*2049 + x0] = T[:, y0:y0+2, x0:x0+2]  ->  [4 corners (dy,dx), C] = 64 B

Grid index 2049 (not 2048) absorbs the float tie u*2048 -> 2048.0 rounding-up
edge case; the extra node row/col 2049 is weight-0.

Device kernel (per core, 256 of 2048 rows, H-sharded 8 ways): per [128 x KK]
pixel block, compute x0 = rint(u*2048-0.5), fx = frac on ACT/DVE, fetch all
128*KK 64-B entries with ONE SWDGE indirect DMA (offset AP [128, KK] -- this
amortizes the ~1 us per-instruction SWDGE cost that dominated the previous
per-column [128,1] design), then a 4-corner weighted sum in fp16 and a single
channel-minor fp16 store per block. Host transposes to [1, C, H, W] fp32.
"""

import numpy as np

C = 8
FULL_H = 2048
FULL_W = 2048
N_CORES = 8
ROWS = FULL_H // N_CORES  # 256
KK = 256                  # pixels per block column chunk

S = 2048                  # collapsed grid density
GRID = S + 1              # 2049 entry rows/cols (x0 in [0, 2048])
NODES = S + 2             # 2050 node rows/cols (corner x0+1 <= 2049)
ENT = 4 * C               # 32 fp16 elems per entry = 64 B

_CACHED = {}


def _build_nodes(texs):
    """T[c, jy, jx] = f(u=jx/S, v=jy/S) for j in [0, NODES). fp32."""
    T = np.zeros((C, NODES, NODES), np.float32)
    j = np.arange(NODES, dtype=np.float64)
    for L, tex in enumerate(texs):
        W = 1024 >> L
        g = j * (W / S) - 0.5
        x0 = np.floor(g).astype(np.int64)
        f = (g - x0).astype(np.float32)

        def interp1d(t, axis):
            t = np.moveaxis(np.asarray(t, np.float32), axis, -1)
            v0 = np.where((x0 >= 0) & (x0 < W), t[..., np.clip(x0, 0, W - 1)], 0.0)
            x1 = x0 + 1
            v1 = np.where((x1 >= 0) & (x1 < W), t[..., np.clip(x1, 0, W - 1)], 0.0)
            return np.moveaxis(v0 * (1.0 - f) + v1 * f, -1, axis)

        T += interp1d(interp1d(tex, 2), 1)
    return T


def _build_table(tex0, tex1, tex2, tex3):
    T = _build_nodes([tex0, tex1, tex2, tex3]).astype(np.float16)
    Tt = T.transpose(1, 2, 0)  # [NODES, NODES, C]
    E = np.empty((GRID, GRID, 4, C), np.float16)
    E[:, :, 0, :] = Tt[0:GRID, 0:GRID]
    E[:, :, 1, :] = Tt[0:GRID, 1:GRID + 1]
    E[:, :, 2, :] = Tt[1:GRID + 1, 0:GRID]
    E[:, :, 3, :] = Tt[1:GRID + 1, 1:GRID + 1]
    return np.ascontiguousarray(E.reshape(GRID * GRID, ENT))


def _build_nc(rows, width, kk):
    import concourse.bacc as bacc
    import concourse.bass as bass
    import concourse.mybir as mybir
    import concourse.tile as tile

    f32 = mybir.dt.float32
    f16 = mybir.dt.float16
    i32 = mybir.dt.int32
    Copy = mybir.ActivationFunctionType.Copy
    MUL = mybir.AluOpType.mult
    ADD = mybir.AluOpType.add
    SUB = mybir.AluOpType.subtract

    nc = bacc.Bacc("TRN2", target_bir_lowering=False, debug=False,
                   num_devices=N_CORES)
    u_d = nc.dram_tensor("u", [rows, width], f32, kind="ExternalInput")
    v_d = nc.dram_tensor("v", [rows, width], f32, kind="ExternalInput")
    tbl_d = nc.dram_tensor("tbl", [GRID * GRID, ENT], f16, kind="ExternalInput")
    out_d = nc.dram_tensor("out", [rows, width, C], f16, kind="ExternalOutput")

    with tile.TileContext(nc) as tc:
        with tc.tile_pool(name="main", bufs=2) as pool:
            for r0 in range(0, rows, 128):
                for w0 in range(0, width, kk):
                    u_t = pool.tile([128, kk], f32, tag="u")
                    v_t = pool.tile([128, kk], f32, tag="v")
                    nc.sync.dma_start(u_t[:], u_d.ap()[r0:r0 + 128, w0:w0 + kk])
                    nc.sync.dma_start(v_t[:], v_d.ap()[r0:r0 + 128, w0:w0 + kk])

                    def cell(src, tagp):
                        """x0 = rint(u*S - 0.5); f = u*S - 0.5 - x0 + 0.5."""
                        s = pool.tile([128, kk], f32, tag=f"s{tagp}")
                        nc.scalar.activation(s[:], src[:], Copy,
                                             bias=-0.5, scale=float(S))
                        ki = pool.tile([128, kk], i32, tag=f"ki{tagp}")
                        nc.vector.tensor_copy(ki[:], s[:])
                        kf = pool.tile([128, kk], f32, tag=f"kf{tagp}")
                        nc.vector.tensor_copy(kf[:], ki[:])
                        fr = pool.tile([128, kk], f32, tag=f"fr{tagp}")
                        nc.vector.scalar_tensor_tensor(
                            out=fr[:], in0=s[:], scalar=0.5, in1=kf[:],
                            op0=ADD, op1=SUB)
                        return kf, fr

                    kx, fx = cell(u_t, "x")
                    ky, fy = cell(v_t, "y")

                    idx = pool.tile([128, kk], i32, tag="idx")
                    nc.vector.scalar_tensor_tensor(
                        out=idx[:], in0=ky[:], scalar=float(GRID),
                        in1=kx[:], op0=MUL, op1=ADD)

                    patch = pool.tile([128, kk * ENT], f16, tag="patch")
                    p3 = patch[:].rearrange("p (k e) -> p k e", e=ENT)
                    nc.gpsimd.indirect_dma_start(
                        out=p3,
                        out_offset=None,
                        in_=tbl_d.ap(),
                        in_offset=bass.IndirectOffsetOnAxis(
                            ap=idx[:, :], axis=0),
                    )

                    gx = pool.tile([128, kk], f32, tag="gx")
                    gy = pool.tile([128, kk], f32, tag="gy")
                    nc.scalar.activation(gx[:], fx[:], Copy, bias=1.0,
                                         scale=-1.0)
                    nc.scalar.activation(gy[:], fy[:], Copy, bias=1.0,
                                         scale=-1.0)
                    w4 = pool.tile([128, 4 * kk], f16, tag="w4")
                    w4v = w4[:].rearrange("p (j k) -> p j k", j=4)
                    nc.vector.tensor_mul(w4v[:, 0, :], gx[:], gy[:])
                    nc.vector.tensor_mul(w4v[:, 1, :], fx[:], gy[:])
                    nc.vector.tensor_mul(w4v[:, 2, :], gx[:], fy[:])
                    nc.vector.tensor_mul(w4v[:, 3, :], fx[:], fy[:])
                    w4b = (w4[:].rearrange("p (j k) -> p j k", j=4)
                           .transpose([0, 2, 1]).unsqueeze(3)
                           .broadcast_to([128, kk, 4, C]))
                    p4 = p3.rearrange("p k (j c) -> p k j c", c=C)
                    nc.vector.tensor_mul(p4, w4b, p4)
                    pv = p3
                    nc.vector.tensor_add(pv[:, :, 0:16], pv[:, :, 0:16],
                                         pv[:, :, 16:32])
                    acc = pool.tile([128, kk * C], f16, tag="acc")
                    accv = acc[:].rearrange("p (k c) -> p k c", c=C)
                    nc.vector.tensor_add(accv, pv[:, :, 0:8], pv[:, :, 8:16])
                    nc.sync.dma_start(
                        out_d.ap()[r0:r0 + 128, w0:w0 + kk, :], acc[:])
    nc.compile()
    return nc


def _get_nc(key, *args):
    if key not in _CACHED:
        _CACHED[key] = _build_nc(*args)
    return _CACHED[key]


def kernel(uv_tensor, iter_nr, tex0, tex1, tex2, tex3):
    from concourse import bass_utils

    bass_utils.upload_artifacts = lambda tmpdir: "local://" + tmpdir

    uv = np.asarray(uv_tensor, dtype=np.float32)
    assert uv.shape == (1, 2, FULL_H, FULL_W), uv.shape
    tbl = _build_table(tex0, tex1, tex2, tex3)

    nc = _get_nc("full", ROWS, FULL_W, KK)

    in_maps = []
    for i in range(N_CORES):
        r0 = i * ROWS
        in_maps.append({
            "u": np.ascontiguousarray(uv[0, 0, r0:r0 + ROWS, :]),
            "v": np.ascontiguousarray(uv[0, 1, r0:r0 + ROWS, :]),
            "tbl": tbl,
        })

    res = bass_utils.run_bass_kernel_spmd(
        nc, in_maps, core_ids=list(range(N_CORES)))
    globals()["_LAST_RES"] = res
    out = np.concatenate(
        [res.results[i]["out"] for i in range(N_CORES)], axis=0)
    # [H, W, C] fp16 -> [1, C, H, W] fp32
    return np.ascontiguousarray(
        out.transpose(2, 0, 1)[None]).astype(np.float32)
